# revision 31
# baseline (speedup 1.0000x reference)
"""LIF (leaky integrate-and-fire) forward recurrence on 8 Trainium2 NeuronCores.

Input  x: (T=16, B=128, N=16384) float32, time-major.
    m[t] = tau * v[t-1] + x[t]
    y[t] = (m[t] >= v_th)            spike, as 0.0/1.0
    v[t] = m[t] * (1 - y[t])         hard reset

Sharding: N split 8 ways (2048 per core); per-neuron recurrence, no
cross-core communication.  Host re-lays each shard as (B, T, N).

Engine split (vs the all-DVE baseline at 88us, which was bound by
32 serial fp32 tensor-tensor DVE ops ~2.29us each):
  - PE (tensor engine, otherwise idle) computes m = xh + xr + tau*v as
    three accumulating identity matmuls per 512-col PSUM bank, all at
    1 cycle/row: xh is fp16, xr is fp8e5m2, v is float32r.
  - ACT: sig = Sign(1 - m) from PSUM -> int8 (the output; host maps
    spike = sig <= 0).
  - DVE: v = (sig > 0) * m -> fp32r SBUF (single PSUM operand; walrus
    forbids two PSUM reads in one op, so the reset mask comes from sig).
Input is compressed host-side to 3 B/elem: x = fp16(x) + e5m2 residual
(exact to ~2^-14); v carries ~12 mantissa bits through fp32r.  Measured
end-to-end l2 error vs the f32 reference is ~5e-3 (a few hundred spike
flips out of 33.5M), well inside the 2e-2 gate; DMA drops from 21MB to
16.8MB per core (~40us floor at ~420GB/s/core).
"""

import numpy as np
import ml_dtypes

import concourse.bass as bass
import concourse.mybir as mybir
from concourse.bass_utils import run_bass_kernel_spmd
from concourse.mybir import AluOpType
from concourse.tile import TileContext

T, B, N = 16, 128, 16384
NCORES = 8
NSH = N // NCORES  # 2048 neurons per core
NB = NSH // 512  # PSUM banks per timestep tile
TAU = 0.5
V_TH = 1.0

IN_CHUNKS = [1, 1, 2, 4, 4, 4]
OUT_CHUNKS = [4, 4, 4, 2, 1, 1]

_cached_nc = None


def _split_multiwaits(nc):
    """Walrus codegen supports only ONE sync-wait per instruction; Tile
    sometimes attaches more.  Move extras onto same-engine NoOps."""
    multi_ok = (mybir.InstEventSemaphore, mybir.InstNoOp)
    for f in nc.m.functions:
        for b in f.blocks:
            new_insts = []
            for inst in b.instructions:
                si = inst.sync_info
                if (
                    not isinstance(inst, multi_ok)
                    and si is not None
                    and len(si.on_wait) > 1
                ):
                    waits = list(si.on_wait)
                    for j, w in enumerate(waits[:-1]):
                        new_insts.append(
                            mybir.InstNoOp(
                                name=f"{inst.name}_presync{j}",
                                engine=inst.engine,
                                sync_info=mybir.SyncInfo(on_wait=[w], on_update=[]),
                            )
                        )
                    inst.sync_info = mybir.SyncInfo(
                        on_wait=[waits[-1]], on_update=list(si.on_update)
                    )
                new_insts.append(inst)
            b.instructions = new_insts


XHB = NSH * 2  # fp16 bytes per step per partition
XRB = NSH  # fp8 bytes per step per partition
XCB = XHB + XRB  # packed row bytes


def _build():
    nc = bass.Bass(trn_type="TRN2")
    # packed input: per (partition, step) row = [xh fp16 bytes | xr fp8 bytes]
    xc = nc.dram_tensor("xc", [B, T, XCB], mybir.dt.uint8, kind="ExternalInput")
    # packed weights: [wI fp16 | wR fp8e5] = 256+128 bytes.  wT stays a
    # typed fp32r tensor: the BIR verifier requires fp32r matmul operands
    # to come from an fp32r-producing instruction (bitcast-of-uint8-DMA
    # does not qualify).
    wc = nc.dram_tensor("wc", [B, 384], mybir.dt.uint8, kind="ExternalInput")
    wT = nc.dram_tensor("wT", [B, B], mybir.dt.float32r, kind="ExternalInput")
    # bank-major output: each per-bank chunk store writes contiguous
    # (cw*512 B) runs per partition instead of 512-B strided fragments
    sig = nc.dram_tensor("y", [B, NB, T, 512], mybir.dt.int8, kind="ExternalOutput")

    with TileContext(nc) as tc:
        with (
            tc.tile_pool(name="sb", bufs=1) as sb,
            tc.tile_pool(name="sgp", bufs=2) as sgp,
            tc.psum_pool(name="ps", bufs=2) as ps,
        ):
            xcs = sb.tile([B, T, XCB], mybir.dt.uint8)
            wcs = sb.tile([B, 384], mybir.dt.uint8)
            wTt = sb.tile([B, B], mybir.dt.float32r)
            wIs = wcs[:, 0:256].bitcast(mybir.dt.float16)
            wRs = wcs[:, 256:384].bitcast(mybir.dt.float8e5)
            wTs = wTt[:]

            # bank-interleaved packed rows: per step, bank b occupies bytes
            # [b*1536, (b+1)*1536) = [xh fp16 (1024B) | xr fp8 (512B)], so a
            # bank's matmuls can start as soon as its own slice lands
            def xh_bank(t, b):
                return xcs[:, t, b * 1536 : b * 1536 + 1024].bitcast(mybir.dt.float16)

            def xr_bank(t, b):
                return xcs[:, t, b * 1536 + 1024 : (b + 1) * 1536].bitcast(
                    mybir.dt.float8e5
                )

            # per-bank state tiles: dependency tracking is per-TILE, so
            # each 512-col bank-chain gets its own tiles to keep the four
            # chains independent (one shared tile serializes all banks)
            vb = [sb.tile([B, 512], mybir.dt.float32r, name=f"v{b}") for b in range(NB)]

            # weights on the scalar ring (free of stores now): they land
            # ~1us after boot, before any x data
            nc.scalar.dma_start(out=wcs[:], in_=wc[:])
            nc.scalar.dma_start(out=wTt[:], in_=wT[:])

            # input stream on the sync ring.  Descriptor generation costs
            # ~0.6-1.3us of SP-sequencer time per dma_start, so the ramp
            # is fine-grained (per-bank for t=0, halves for t=1) and the
            # rest is one packed DMA per step: x_mms(t) gate on their own
            # step (bank) only and ride the stream with no chunk stalls.
            for b in range(NB):
                bs = slice(b * 1536, (b + 1) * 1536)
                nc.sync.dma_start(out=xcs[:, 0, bs], in_=xc[:, 0, bs])
            for h in range(2):
                hs = slice(h * 3072, (h + 1) * 3072)
                nc.sync.dma_start(out=xcs[:, 1, hs], in_=xc[:, 1, hs])
            for t in range(2, T):
                nc.sync.dma_start(out=xcs[:, t : t + 1, :], in_=xc[:, t : t + 1, :])

            # output chunking: (start, width) per chunk, and for each t the
            # chunk it belongs to
            chunks = []
            t0 = 0
            for w in OUT_CHUNKS:
                chunks.append((t0, w))
                t0 += w
            chunk_of = {}
            for ci, (t0, w) in enumerate(chunks):
                for t in range(t0, t0 + w):
                    chunk_of[t] = ci

            # per-bank PSUM m tiles (4 tags x 2 bufs = all 8 banks)
            mt = {}  # (t, b) -> psum tile

            def m_tile(t, b):
                if (t, b) not in mt:
                    mt[(t, b)] = ps.tile(
                        [B, 512], mybir.dt.float32, tag=f"m{b}", name=f"m{t}_{b}"
                    )
                return mt[(t, b)]

            # per-bank, per-chunk sg tiles (double-buffered so the chunk
            # store never WAR-blocks the next chunk's sig writes)
            sgt = {}  # (ci, b) -> sbuf int8 tile [B, w, 512]

            def sg_tile(t, b):
                ci = chunk_of[t]
                if (ci, b) not in sgt:
                    sgt[(ci, b)] = sgp.tile(
                        [B, 4, 512], mybir.dt.int8, tag=f"sg{b}", name=f"sg{ci}_{b}"
                    )
                return sgt[(ci, b)]

            # PE warmup: dummy self-contained matmuls on a junk tile
            # (memset by DVE at t=0, so no DMA wait).  N=512 x 10 gives
            # >4us of sustained PE activity: the HAM clock gate needs
            # ~3.4us busy to un-throttle 1.2 -> 2.4 GHz, and must flip
            # before the real chain starts.  They scribble on m(0,0); the
            # real x-matmul resets it (start=True clears has_written).
            junk = sb.tile([B, 512], mybir.dt.bfloat16, name="junk")
            nc.vector.memset(junk[:], 0.0)
            for _ in range(2):
                nc.tensor.matmul(
                    out=m_tile(0, 0)[:], lhsT=junk[:, :B], rhs=junk[:],
                    start=True, stop=True, skip_group_check=True,
                )

            def x_mms(t):
                # per-bank xh/xr pairs: bank b's accumulation closes as
                # soon as its own input slice lands
                for b in range(NB):
                    nc.tensor.matmul(
                        out=m_tile(t, b)[:], lhsT=wIs, rhs=xh_bank(t, b),
                        start=True, stop=False,
                    )
                    nc.tensor.matmul(
                        out=m_tile(t, b)[:], lhsT=wRs, rhs=xr_bank(t, b),
                        start=False, stop=(t == 0),
                    )

            # The serial chain per bank b is v_mm(t,b) -> sig(t,b) ->
            # vop(t,b) -> v_mm(t+1,b); the four bank-chains are fully
            # independent (separate m/v/sg tiles) and pipeline round-robin
            # across PE/ACT/DVE.  x-matmuls for t+1 are emitted after
            # v_mms(t): they refill the PE during the sig/vop latency.
            x_mms(0)
            for t in range(T):
                ci = chunk_of[t]
                c0, cw = chunks[ci]
                for b in range(NB):
                    cs = slice(b * 512, (b + 1) * 512)
                    if t > 0:
                        nc.tensor.matmul(
                            out=m_tile(t, b)[:], lhsT=wTs, rhs=vb[b][:],
                            start=False, stop=True,
                        )
                    nc.scalar.activation(
                        sg_tile(t, b)[:, t - c0, :], m_tile(t, b)[:],
                        mybir.ActivationFunctionType.Sign,
                        bias=V_TH, scale=-1.0,
                    )
                    if t < T - 1:
                        nc.vector.scalar_tensor_tensor(
                            vb[b][:], sg_tile(t, b)[:, t - c0, :], 0, m_tile(t, b)[:],
                            AluOpType.is_gt, AluOpType.mult,
                        )
                if t + 1 < T:
                    x_mms(t + 1)
                if t == c0 + cw - 1:
                    # chunk finished: store each bank's sg tile via the
                    # SWDGE (gpsimd) path -- descriptor generation runs on
                    # the idle Q7 cores instead of stalling the ACT queue
                    for b in range(NB):
                        nc.gpsimd.dma_start(
                            out=sig[:, b, c0 : c0 + cw, :], in_=sgt[(ci, b)][:, :cw, :]
                        )
    _split_multiwaits(nc)
    return nc


def kernel(x: np.ndarray) -> np.ndarray:
    global _cached_nc
    if _cached_nc is None:
        _cached_nc = _build()
    nc = _cached_nc

    x = np.ascontiguousarray(x, dtype=np.float32)
    assert x.shape == (T, B, N)
    # (T, B, N) -> per-core (B, T, NSH) shards; split x = fp16 + e5m2 residual
    xbt = np.ascontiguousarray(x.transpose(1, 0, 2))
    xh = xbt.astype(np.float16)
    xr = (xbt - xh.astype(np.float32)).astype(ml_dtypes.float8_e5m2)
    # packed weights row: [wI fp16 | wR fp8e5] bytes; wT separate (fp32r)
    wc = np.concatenate(
        [
            np.eye(B, dtype=np.float16).view(np.uint8),
            np.eye(B, dtype=ml_dtypes.float8_e5m2).view(np.uint8),
        ],
        axis=1,
    )
    wT = (TAU * np.eye(B)).astype(np.float32)
    in_maps = []
    for k in range(NCORES):
        ns = slice(k * NSH, (k + 1) * NSH)
        # bank-interleaved packed rows: per step, per 512-col bank:
        # [xh fp16 bytes (1024) | xr fp8 bytes (512)]
        xhk = np.ascontiguousarray(xh[:, :, ns]).view(np.uint8).reshape(B, T, NB, 1024)
        xrk = np.ascontiguousarray(xr[:, :, ns]).view(np.uint8).reshape(B, T, NB, 512)
        xck = np.concatenate([xhk, xrk], axis=3).reshape(B, T, XCB)
        in_maps.append({"xc": xck, "wc": wc, "wT": wT})
    res = run_bass_kernel_spmd(nc, in_maps, core_ids=list(range(NCORES)))
    global _last_exec_ns
    if res.exec_time_ns is not None:
        _last_exec_ns = res.exec_time_ns
    # per-core int8 sign, bank-major (B, NB, T, 512): sig = Sign(1-m),
    # spike <=> sig <= 0.  Un-permute banks then cores then time-major.
    outs = [
        r["y"].transpose(0, 2, 1, 3).reshape(B, T, NSH) for r in res.results
    ]
    out = np.concatenate(outs, axis=2)
    return (
        np.ascontiguousarray(out.transpose(1, 0, 2)) <= 0
    ).astype(np.float32)


_last_exec_ns = None


# revision 34
# speedup vs baseline: 1.0870x; 1.0870x over previous
"""LIF (leaky integrate-and-fire) forward recurrence on 8 Trainium2 NeuronCores.

Input  x: (T=16, B=128, N=16384) float32, time-major.
    m[t] = tau * v[t-1] + x[t]
    y[t] = (m[t] >= v_th)            spike, as 0.0/1.0
    v[t] = m[t] * (1 - y[t])         hard reset

Sharding: N split 8 ways (2048 per core); per-neuron recurrence, no
cross-core communication.  Host re-lays each shard as (B, T, N).

Engine split (vs the all-DVE baseline at 88us, which was bound by
32 serial fp32 tensor-tensor DVE ops ~2.29us each):
  - PE (tensor engine, otherwise idle) computes m = xh + xr + tau*v as
    three accumulating identity matmuls per 512-col PSUM bank, all at
    1 cycle/row: xh is fp16, xr is fp8e5m2, v is float32r.
  - ACT: sig = Sign(1 - m) from PSUM -> int8 (the output; host maps
    spike = sig <= 0).
  - DVE: v = (sig > 0) * m -> fp32r SBUF (single PSUM operand; walrus
    forbids two PSUM reads in one op, so the reset mask comes from sig).
Input is compressed host-side to 3 B/elem: x = fp16(x) + e5m2 residual
(exact to ~2^-14); v carries ~12 mantissa bits through fp32r.  Measured
end-to-end l2 error vs the f32 reference is ~5e-3 (a few hundred spike
flips out of 33.5M), well inside the 2e-2 gate; DMA drops from 21MB to
16.8MB per core (~40us floor at ~420GB/s/core).
"""

import numpy as np
import ml_dtypes

import concourse.bass as bass
import concourse.mybir as mybir
from concourse.bass_utils import run_bass_kernel_spmd
from concourse.mybir import AluOpType
from concourse.tile import TileContext

T, B, N = 16, 128, 16384
NCORES = 8
NSH = N // NCORES  # 2048 neurons per core
NB = NSH // 512  # PSUM banks per timestep tile
TAU = 0.5
V_TH = 1.0

IN_CHUNKS = [1, 1, 2, 4, 4, 4]
OUT_CHUNKS = [4, 4, 4, 2, 1, 1]

_cached_nc = None


def _split_multiwaits(nc):
    """Walrus codegen supports only ONE sync-wait per instruction; Tile
    sometimes attaches more.  Move extras onto same-engine NoOps."""
    multi_ok = (mybir.InstEventSemaphore, mybir.InstNoOp)
    for f in nc.m.functions:
        for b in f.blocks:
            new_insts = []
            for inst in b.instructions:
                si = inst.sync_info
                if (
                    not isinstance(inst, multi_ok)
                    and si is not None
                    and len(si.on_wait) > 1
                ):
                    waits = list(si.on_wait)
                    for j, w in enumerate(waits[:-1]):
                        new_insts.append(
                            mybir.InstNoOp(
                                name=f"{inst.name}_presync{j}",
                                engine=inst.engine,
                                sync_info=mybir.SyncInfo(on_wait=[w], on_update=[]),
                            )
                        )
                    inst.sync_info = mybir.SyncInfo(
                        on_wait=[waits[-1]], on_update=list(si.on_update)
                    )
                new_insts.append(inst)
            b.instructions = new_insts


XHB = NSH * 2  # fp16 bytes per step per partition
XRB = NSH  # fp8 bytes per step per partition
XCB = XHB + XRB  # packed row bytes


def _build():
    nc = bass.Bass(trn_type="TRN2")
    # packed input: per (partition, step) row = [xh fp16 bytes | xr fp8 bytes]
    xc = nc.dram_tensor("xc", [B, T, XCB], mybir.dt.uint8, kind="ExternalInput")
    # packed weights: [wI fp16 | wR fp8e5] = 256+128 bytes.  wT stays a
    # typed fp32r tensor: the BIR verifier requires fp32r matmul operands
    # to come from an fp32r-producing instruction (bitcast-of-uint8-DMA
    # does not qualify).
    wc = nc.dram_tensor("wc", [B, 384], mybir.dt.uint8, kind="ExternalInput")
    wT = nc.dram_tensor("wT", [B, B], mybir.dt.float32r, kind="ExternalInput")
    # bank-major output: each per-bank chunk store writes contiguous
    # (cw*512 B) runs per partition instead of 512-B strided fragments
    sig = nc.dram_tensor("y", [B, NB, T, 512], mybir.dt.int8, kind="ExternalOutput")

    with TileContext(nc) as tc:
        with (
            tc.tile_pool(name="sb", bufs=1) as sb,
            tc.tile_pool(name="sgp", bufs=2) as sgp,
            tc.psum_pool(name="ps", bufs=2) as ps,
        ):
            xcs = sb.tile([B, T, XCB], mybir.dt.uint8)
            wcs = sb.tile([B, 384], mybir.dt.uint8)
            wTt = sb.tile([B, B], mybir.dt.float32r)
            wIs = wcs[:, 0:256].bitcast(mybir.dt.float16)
            wRs = wcs[:, 256:384].bitcast(mybir.dt.float8e5)
            wTs = wTt[:]

            # bank-interleaved packed rows: per step, bank b occupies bytes
            # [b*1536, (b+1)*1536) = [xh fp16 (1024B) | xr fp8 (512B)], so a
            # bank's matmuls can start as soon as its own slice lands
            def xh_bank(t, b):
                return xcs[:, t, b * 1536 : b * 1536 + 1024].bitcast(mybir.dt.float16)

            def xr_bank(t, b):
                return xcs[:, t, b * 1536 + 1024 : (b + 1) * 1536].bitcast(
                    mybir.dt.float8e5
                )

            # per-bank state tiles: dependency tracking is per-TILE, so
            # each 512-col bank-chain gets its own tiles to keep the four
            # chains independent (one shared tile serializes all banks)
            vb = [sb.tile([B, 512], mybir.dt.float32r, name=f"v{b}") for b in range(NB)]

            # weights on the scalar ring (free of stores now): they land
            # ~1us after boot, before any x data
            nc.scalar.dma_start(out=wcs[:], in_=wc[:])
            nc.scalar.dma_start(out=wTt[:], in_=wT[:])

            # input stream on the sync ring, one packed DMA per step:
            # x_mms(t) gate on their own step only and ride the stream
            # (finer ramps buy nothing: the first ~4.5us of PE time go to
            # the HAM warmup anyway).
            for t in range(T):
                nc.sync.dma_start(out=xcs[:, t : t + 1, :], in_=xc[:, t : t + 1, :])

            # output chunking: (start, width) per chunk, and for each t the
            # chunk it belongs to
            chunks = []
            t0 = 0
            for w in OUT_CHUNKS:
                chunks.append((t0, w))
                t0 += w
            chunk_of = {}
            for ci, (t0, w) in enumerate(chunks):
                for t in range(t0, t0 + w):
                    chunk_of[t] = ci

            # per-bank PSUM m tiles (4 tags x 2 bufs = all 8 banks)
            mt = {}  # (t, b) -> psum tile

            def m_tile(t, b):
                if (t, b) not in mt:
                    mt[(t, b)] = ps.tile(
                        [B, 512], mybir.dt.float32, tag=f"m{b}", name=f"m{t}_{b}"
                    )
                return mt[(t, b)]

            # per-bank, per-chunk sg tiles (double-buffered so the chunk
            # store never WAR-blocks the next chunk's sig writes)
            sgt = {}  # (ci, b) -> sbuf int8 tile [B, w, 512]

            def sg_tile(t, b):
                ci = chunk_of[t]
                if (ci, b) not in sgt:
                    sgt[(ci, b)] = sgp.tile(
                        [B, 4, 512], mybir.dt.int8, tag=f"sg{b}", name=f"sg{ci}_{b}"
                    )
                return sgt[(ci, b)]

            # PE warmup: dummy self-contained matmuls on a junk tile
            # (memset by DVE at t=0, so no DMA wait).  N=512 x 10 gives
            # >4us of sustained PE activity: the HAM clock gate needs
            # ~3.4us busy to un-throttle 1.2 -> 2.4 GHz, and must flip
            # before the real chain starts.  They scribble on m(0,0); the
            # real x-matmul resets it (start=True clears has_written).
            junk = sb.tile([B, 512], mybir.dt.bfloat16, name="junk")
            nc.vector.memset(junk[:], 0.0)
            for _ in range(10):
                nc.tensor.matmul(
                    out=m_tile(0, 0)[:], lhsT=junk[:, :B], rhs=junk[:],
                    start=True, stop=True, skip_group_check=True,
                )

            def x_mms(t):
                # per-bank xh/xr pairs: bank b's accumulation closes as
                # soon as its own input slice lands
                for b in range(NB):
                    nc.tensor.matmul(
                        out=m_tile(t, b)[:], lhsT=wIs, rhs=xh_bank(t, b),
                        start=True, stop=False,
                    )
                    nc.tensor.matmul(
                        out=m_tile(t, b)[:], lhsT=wRs, rhs=xr_bank(t, b),
                        start=False, stop=(t == 0),
                    )

            # The serial chain per bank b is v_mm(t,b) -> sig(t,b) ->
            # vop(t,b) -> v_mm(t+1,b); the four bank-chains are fully
            # independent (separate m/v/sg tiles) and pipeline round-robin
            # across PE/ACT/DVE.  x-matmuls for t+1 are emitted after
            # v_mms(t): they refill the PE during the sig/vop latency.
            x_mms(0)
            for t in range(T):
                ci = chunk_of[t]
                c0, cw = chunks[ci]
                for b in range(NB):
                    cs = slice(b * 512, (b + 1) * 512)
                    if t > 0:
                        nc.tensor.matmul(
                            out=m_tile(t, b)[:], lhsT=wTs, rhs=vb[b][:],
                            start=False, stop=True,
                        )
                    nc.scalar.activation(
                        sg_tile(t, b)[:, t - c0, :], m_tile(t, b)[:],
                        mybir.ActivationFunctionType.Sign,
                        bias=V_TH, scale=-1.0,
                    )
                    if t < T - 1:
                        nc.vector.scalar_tensor_tensor(
                            vb[b][:], sg_tile(t, b)[:, t - c0, :], 0, m_tile(t, b)[:],
                            AluOpType.is_gt, AluOpType.mult,
                        )
                    if cw == 1:
                        # tail chunks (width 1): store each bank right
                        # after its sig, split across both DGE queues so
                        # the trailing descriptor-gens run in parallel
                        eng = nc.gpsimd if b < 2 else nc.scalar
                        eng.dma_start(
                            out=sig[:, b, c0 : c0 + 1, :], in_=sgt[(ci, b)][:, :1, :]
                        )
                if t + 1 < T:
                    x_mms(t + 1)
                if t == c0 + cw - 1 and cw > 1:
                    # chunk finished: store each bank's sg tile via the
                    # SWDGE (gpsimd) path -- descriptor generation runs on
                    # the idle Q7 cores instead of stalling the ACT queue
                    for b in range(NB):
                        nc.gpsimd.dma_start(
                            out=sig[:, b, c0 : c0 + cw, :], in_=sgt[(ci, b)][:, :cw, :]
                        )
    _split_multiwaits(nc)
    return nc


def kernel(x: np.ndarray) -> np.ndarray:
    global _cached_nc
    if _cached_nc is None:
        _cached_nc = _build()
    nc = _cached_nc

    x = np.ascontiguousarray(x, dtype=np.float32)
    assert x.shape == (T, B, N)
    # (T, B, N) -> per-core (B, T, NSH) shards; split x = fp16 + e5m2 residual
    xbt = np.ascontiguousarray(x.transpose(1, 0, 2))
    xh = xbt.astype(np.float16)
    xr = (xbt - xh.astype(np.float32)).astype(ml_dtypes.float8_e5m2)
    # packed weights row: [wI fp16 | wR fp8e5] bytes; wT separate (fp32r)
    wc = np.concatenate(
        [
            np.eye(B, dtype=np.float16).view(np.uint8),
            np.eye(B, dtype=ml_dtypes.float8_e5m2).view(np.uint8),
        ],
        axis=1,
    )
    wT = (TAU * np.eye(B)).astype(np.float32)
    in_maps = []
    for k in range(NCORES):
        ns = slice(k * NSH, (k + 1) * NSH)
        # bank-interleaved packed rows: per step, per 512-col bank:
        # [xh fp16 bytes (1024) | xr fp8 bytes (512)]
        xhk = np.ascontiguousarray(xh[:, :, ns]).view(np.uint8).reshape(B, T, NB, 1024)
        xrk = np.ascontiguousarray(xr[:, :, ns]).view(np.uint8).reshape(B, T, NB, 512)
        xck = np.concatenate([xhk, xrk], axis=3).reshape(B, T, XCB)
        in_maps.append({"xc": xck, "wc": wc, "wT": wT})
    res = run_bass_kernel_spmd(nc, in_maps, core_ids=list(range(NCORES)))
    global _last_exec_ns
    if res.exec_time_ns is not None:
        _last_exec_ns = res.exec_time_ns
    # per-core int8 sign, bank-major (B, NB, T, 512): sig = Sign(1-m),
    # spike <=> sig <= 0.  Un-permute banks then cores then time-major.
    outs = [
        r["y"].transpose(0, 2, 1, 3).reshape(B, T, NSH) for r in res.results
    ]
    out = np.concatenate(outs, axis=2)
    return (
        np.ascontiguousarray(out.transpose(1, 0, 2)) <= 0
    ).astype(np.float32)


_last_exec_ns = None
